# revision 1
# baseline (speedup 1.0000x reference)
"""Causal sliding-window attention (W=128) for Trainium2, 8 NeuronCores.

Problem: B=4, T=4096, D=1024, H=16, HD=64, window W=128 (incl. self).
  Q = x@Wq+bq; K = x@Wk+bk; V = x@Wv+bv  (per head hd=64)
  scores = QK^T/sqrt(hd) with banded causal-window mask, softmax
  context = attn @ V            (output 2)
  output = context @ Wo + bo    (output 1)

Sharding: 8 cores = (batch b in 0..3) x (sequence half hh in 0..1).
Each core owns 2048 tokens and receives a W-token halo on the left so it
can compute the halo K/V itself (zeros for the first block; masked out).
Host side only pads/transposes/casts inputs and concatenates outputs.

Per-core kernel (all intermediates stay in SBUF):
  phase 1: QT,KT feature-major [D, tok] bf16; V token-major [tok, D] fp16
           with an interleaved ones-column per head ([tok, 16*65]) so the
           attention AV matmul also produces the softmax denominator.
  phase 2: per (qblock, head): scoresT [keys,q] = KT_slice.T-matmul,
           exp on ACT, 0/1 band-mask mul on DVE, AV+denominator matmul,
           reciprocal + per-partition normalize into token-major CTX.
  phase 3: per qblock: PE-transpose CTX -> CTXT bf16, out-proj matmul
           with rank-1 bias fold, DMA out (output + context).

The attention scale 1/sqrt(64) is folded into Wq/bq on the host.
"""

import numpy as np
import ml_dtypes
from contextlib import ExitStack

import concourse.tile as tile
from concourse import bacc, mybir
from concourse.bass_utils import run_bass_kernel_spmd
from concourse.masks import make_identity

B, T, D = 4, 4096, 1024
H, W, HD = 16, 128, 64
NCORES = 8
TOWN = T // 2          # tokens owned per core = 2048
TH = TOWN + W          # with halo = 2176
NQB = TOWN // W        # 16 query blocks per core
NKT = TH // W          # 17 key token-tiles per core
P = 128

F32 = mybir.dt.float32
F16 = mybir.dt.float16
BF16 = mybir.dt.bfloat16

_CACHE = {}


def _build_program():
    nc = bacc.Bacc("TRN2", target_bir_lowering=False, debug=False,
                   num_devices=NCORES)

    xt = nc.dram_tensor("xt", [D, TH], BF16, kind="ExternalInput").ap()
    wq = nc.dram_tensor("wq", [D, D], BF16, kind="ExternalInput").ap()
    wk = nc.dram_tensor("wk", [D, D], BF16, kind="ExternalInput").ap()
    wv = nc.dram_tensor("wv", [D, D], BF16, kind="ExternalInput").ap()
    wo = nc.dram_tensor("wo", [D, D], BF16, kind="ExternalInput").ap()
    bqt = nc.dram_tensor("bqt", [P, 8], F32, kind="ExternalInput").ap()
    bkt = nc.dram_tensor("bkt", [P, 8], F32, kind="ExternalInput").ap()
    bvr = nc.dram_tensor("bvr", [1, D], BF16, kind="ExternalInput").ap()
    bor = nc.dram_tensor("bor", [1, D], BF16, kind="ExternalInput").ap()
    mt0f = nc.dram_tensor("mt0f", [P, P], F16, kind="ExternalInput").ap()
    mt0 = nc.dram_tensor("mt0", [P, P], F16, kind="ExternalInput").ap()
    mt1 = nc.dram_tensor("mt1", [P, P], F16, kind="ExternalInput").ap()

    outp = nc.dram_tensor("outp", [TOWN, D], F32, kind="ExternalOutput").ap()
    ctxp = nc.dram_tensor("ctxp", [TOWN, D], F32, kind="ExternalOutput").ap()

    AF = mybir.ActivationFunctionType

    with tile.TileContext(nc) as tc:
        with ExitStack() as ctx:
            xt_p = ctx.enter_context(tc.tile_pool(name="xt_p", bufs=1))
            qt_p = ctx.enter_context(tc.tile_pool(name="qt_p", bufs=1))
            kt_p = ctx.enter_context(tc.tile_pool(name="kt_p", bufs=1))
            v_p = ctx.enter_context(tc.tile_pool(name="v_p", bufs=1))
            w_p = ctx.enter_context(tc.tile_pool(name="w_p", bufs=8))
            wo_p = ctx.enter_context(tc.tile_pool(name="wo_p", bufs=1))
            ctx_p = ctx.enter_context(tc.tile_pool(name="ctx_p", bufs=2))
            ctxt_p = ctx.enter_context(tc.tile_pool(name="ctxt_p", bufs=2))
            pr_p = ctx.enter_context(tc.tile_pool(name="pr_p", bufs=3))
            out_p = ctx.enter_context(tc.tile_pool(name="out_p", bufs=2))
            const_p = ctx.enter_context(tc.tile_pool(name="const_p", bufs=1))
            rc_p = ctx.enter_context(tc.tile_pool(name="rc_p", bufs=4))
            psum = ctx.enter_context(
                tc.tile_pool(name="psum", bufs=8, space="PSUM"))

            # ---- constants / inputs ----
            xt_sb = []
            for k in range(8):
                t = xt_p.tile([P, TH], BF16, tag=f"xt{k}", name=f"xt{k}")
                nc.sync.dma_start(t[:], xt[k * P:(k + 1) * P, :])
                xt_sb.append(t)

            bqt_sb = const_p.tile([P, 8], F32, tag="bqt", name="bqt_sb")
            nc.sync.dma_start(bqt_sb[:], bqt[:])
            bkt_sb = const_p.tile([P, 8], F32, tag="bkt", name="bkt_sb")
            nc.sync.dma_start(bkt_sb[:], bkt[:])
            bvr_sb = const_p.tile([1, D], BF16, tag="bvr", name="bvr_sb")
            nc.sync.dma_start(bvr_sb[:], bvr[:])
            bor_sb = const_p.tile([1, D], BF16, tag="bor", name="bor_sb")
            nc.sync.dma_start(bor_sb[:], bor[:])
            m0f_sb = const_p.tile([P, P], F16, tag="m0f", name="m0f_sb")
            nc.sync.dma_start(m0f_sb[:], mt0f[:])
            m0_sb = const_p.tile([P, P], F16, tag="m0", name="m0_sb")
            nc.sync.dma_start(m0_sb[:], mt0[:])
            m1_sb = const_p.tile([P, P], F16, tag="m1", name="m1_sb")
            nc.sync.dma_start(m1_sb[:], mt1[:])
            ones_sb = const_p.tile([1, P], BF16, tag="ones", name="ones_sb")
            nc.vector.memset(ones_sb[:], 1.0)
            ident = const_p.tile([P, P], F32, tag="ident", name="ident")
            make_identity(nc, ident)

            # ---- phase 1a: QT, KT (feature-major) ----
            qt_sb = [qt_p.tile([P, TOWN], BF16, tag=f"qt{m}", name=f"qt{m}")
                     for m in range(8)]
            kt_sb = [kt_p.tile([P, TH], BF16, tag=f"kt{m}", name=f"kt{m}")
                     for m in range(8)]

            wq_sb = []
            for k in range(8):
                t = w_p.tile([P, D], BF16, tag="w", name=f"wq{k}")
                nc.sync.dma_start(t[:], wq[k * P:(k + 1) * P, :])
                wq_sb.append(t)
            # Q: tokens are xt cols [W, TH) -> 4 chunks of 512
            for m in range(8):
                for n in range(4):
                    ps = psum.tile([P, 512], F32, tag="pp", name=f"q{m}{n}")
                    for k in range(8):
                        nc.tensor.matmul(
                            ps[:],
                            wq_sb[k][:, m * P:(m + 1) * P],
                            xt_sb[k][:, W + n * 512:W + (n + 1) * 512],
                            start=(k == 0), stop=(k == 7))
                    nc.scalar.activation(
                        qt_sb[m][:, n * 512:(n + 1) * 512], ps[:],
                        AF.Identity, bias=bqt_sb[:, m:m + 1])

            wk_sb = []
            for k in range(8):
                t = w_p.tile([P, D], BF16, tag="w", name=f"wk{k}")
                nc.sync.dma_start(t[:], wk[k * P:(k + 1) * P, :])
                wk_sb.append(t)
            # K: all TH tokens -> chunks [512,512,512,512,128]
            kchunks = [(0, 512), (512, 512), (1024, 512), (1536, 512),
                       (2048, 128)]
            for m in range(8):
                for (c0, cn) in kchunks:
                    ps = psum.tile([P, 512], F32, tag="pp", name=f"k{m}{c0}")
                    for k in range(8):
                        nc.tensor.matmul(
                            ps[:, 0:cn],
                            wk_sb[k][:, m * P:(m + 1) * P],
                            xt_sb[k][:, c0:c0 + cn],
                            start=(k == 0), stop=(k == 7))
                    nc.scalar.activation(
                        kt_sb[m][:, c0:c0 + cn], ps[:, 0:cn],
                        AF.Identity, bias=bkt_sb[:, m:m + 1])

            # ---- phase 1b: V token-major with interleaved ones columns ----
            wv_sb = []
            for k in range(8):
                t = w_p.tile([P, D], BF16, tag="w", name=f"wv{k}")
                nc.sync.dma_start(t[:], wv[k * P:(k + 1) * P, :])
                wv_sb.append(t)
            v_sb = []
            for ti in range(NKT):
                vt = v_p.tile([P, H * (HD + 1)], F16, tag=f"v{ti}",
                              name=f"v{ti}")
                vview = vt[:].rearrange("p (g c) -> p g c", c=HD + 1)
                nc.vector.memset(vview[:, :, HD:HD + 1], 1.0)
                for n2 in range(2):
                    ps = psum.tile([P, 512], F32, tag="pp", name=f"v{ti}{n2}")
                    for k in range(8):
                        nc.tensor.matmul(
                            ps[:],
                            xt_sb[k][:, ti * P:(ti + 1) * P],
                            wv_sb[k][:, n2 * 512:(n2 + 1) * 512],
                            start=(k == 0), stop=False)
                    nc.tensor.matmul(
                        ps[:], ones_sb[:],
                        bvr_sb[:, n2 * 512:(n2 + 1) * 512],
                        start=False, stop=True)
                    psv = ps[:].rearrange("p (g c) -> p g c", c=HD)
                    nc.vector.tensor_copy(
                        vview[:, n2 * 8:(n2 + 1) * 8, 0:HD], psv[:])
                v_sb.append(vt)

            # ---- wo tiles (resident) ----
            wo_sb = []
            for k in range(8):
                t = wo_p.tile([P, D], BF16, tag=f"wo{k}", name=f"wo{k}")
                nc.sync.dma_start(t[:], wo[k * P:(k + 1) * P, :])
                wo_sb.append(t)

            # ---- phase 2+3: attention per qblock, then out-proj ----
            for qb in range(NQB):
                ctx_t = ctx_p.tile([P, D], F32, tag="ctx", name=f"ctx{qb}")
                for h in range(H):
                    kq = h // 2
                    off = (h % 2) * HD
                    qrhs = qt_sb[kq][off:off + HD, qb * P:(qb + 1) * P]
                    s0 = psum.tile([P, P], F32, tag="pp", name=f"s0_{qb}_{h}")
                    nc.tensor.matmul(
                        s0[:], kt_sb[kq][off:off + HD, qb * P:(qb + 1) * P],
                        qrhs, start=True, stop=True)
                    s1 = psum.tile([P, P], F32, tag="pp", name=f"s1_{qb}_{h}")
                    nc.tensor.matmul(
                        s1[:],
                        kt_sb[kq][off:off + HD, (qb + 1) * P:(qb + 2) * P],
                        qrhs, start=True, stop=True)
                    e0 = pr_p.tile([P, P], F16, tag="e0", name=f"e0_{qb}_{h}")
                    nc.scalar.activation(e0[:], s0[:], AF.Exp)
                    e1 = pr_p.tile([P, P], F16, tag="e1", name=f"e1_{qb}_{h}")
                    nc.scalar.activation(e1[:], s1[:], AF.Exp)
                    em0 = pr_p.tile([P, P], F16, tag="em0",
                                    name=f"em0_{qb}_{h}")
                    nc.vector.tensor_mul(
                        em0[:], e0[:], m0f_sb[:] if qb == 0 else m0_sb[:])
                    em1 = pr_p.tile([P, P], F16, tag="em1",
                                    name=f"em1_{qb}_{h}")
                    nc.vector.tensor_mul(em1[:], e1[:], m1_sb[:])
                    c = psum.tile([P, HD + 1], F32, tag="pp",
                                  name=f"c_{qb}_{h}")
                    nc.tensor.matmul(
                        c[:], em0[:],
                        v_sb[qb][:, h * (HD + 1):(h + 1) * (HD + 1)],
                        start=True, stop=False)
                    nc.tensor.matmul(
                        c[:], em1[:],
                        v_sb[qb + 1][:, h * (HD + 1):(h + 1) * (HD + 1)],
                        start=False, stop=True)
                    rc = rc_p.tile([P, 1], F32, tag="rc", name=f"rc_{qb}_{h}")
                    nc.vector.reciprocal(rc[:], c[:, HD:HD + 1])
                    nc.vector.tensor_scalar_mul(
                        ctx_t[:, h * HD:(h + 1) * HD], c[:, 0:HD], rc[:])

                # transpose ctx -> ctxt (bf16) and out-projection
                ctxt = []
                for dd in range(8):
                    tp = psum.tile([P, P], F32, tag="pp", name=f"tp{qb}{dd}")
                    nc.tensor.transpose(
                        tp[:], ctx_t[:, dd * P:(dd + 1) * P], ident[:])
                    ct = ctxt_p.tile([P, P], BF16, tag=f"ctxt{dd}",
                                     name=f"ctxt{qb}{dd}")
                    nc.vector.tensor_copy(ct[:], tp[:])
                    ctxt.append(ct)
                out_sb = out_p.tile([P, D], F32, tag="out", name=f"out{qb}")
                for n2 in range(2):
                    po = psum.tile([P, 512], F32, tag="pp", name=f"po{qb}{n2}")
                    for dd in range(8):
                        nc.tensor.matmul(
                            po[:], ctxt[dd][:],
                            wo_sb[dd][:, n2 * 512:(n2 + 1) * 512],
                            start=(dd == 0), stop=False)
                    nc.tensor.matmul(
                        po[:], ones_sb[:], bor_sb[:, n2 * 512:(n2 + 1) * 512],
                        start=False, stop=True)
                    nc.vector.tensor_copy(
                        out_sb[:, n2 * 512:(n2 + 1) * 512], po[:])
                nc.sync.dma_start(outp[qb * P:(qb + 1) * P, :], out_sb[:])
                nc.sync.dma_start(ctxp[qb * P:(qb + 1) * P, :], ctx_t[:])

    nc.compile()
    return nc


def _prep_inputs(x, Wq, bq, Wk, bk, Wv, bv, Wo, bo):
    """Build the 8 per-core input maps (host-side shard/pad/cast)."""
    f32 = np.float32
    x = np.asarray(x, f32)
    scale = f32(1.0 / np.sqrt(HD))
    wq_s = (np.asarray(Wq, f32) * scale).astype(ml_dtypes.bfloat16)
    bq_s = (np.asarray(bq, f32) * scale)
    wk_b = np.asarray(Wk, f32).astype(ml_dtypes.bfloat16)
    wv_b = np.asarray(Wv, f32).astype(ml_dtypes.bfloat16)
    wo_b = np.asarray(Wo, f32).astype(ml_dtypes.bfloat16)
    bqt = np.ascontiguousarray(bq_s.reshape(8, P).T)
    bkt = np.ascontiguousarray(np.asarray(bk, f32).reshape(8, P).T)
    bvr = np.asarray(bv, f32).reshape(1, D).astype(ml_dtypes.bfloat16)
    bor = np.asarray(bo, f32).reshape(1, D).astype(ml_dtypes.bfloat16)

    # x padded with a leading W zeros along T, then per-core transposed slice
    xp = np.zeros((B, T + W, D), f32)
    xp[:, W:] = x

    r = np.arange(P)
    band0 = (r[:, None] > r[None, :]).astype(np.float16)   # keys tile 0: p > r
    band1 = (r[:, None] <= r[None, :]).astype(np.float16)  # keys tile 1: p <= r
    zeros0 = np.zeros((P, P), np.float16)

    in_maps = []
    for c in range(NCORES):
        b, hh = c // 2, c % 2
        t0 = hh * TOWN
        xt_c = np.ascontiguousarray(
            xp[b, t0:t0 + TH].T).astype(ml_dtypes.bfloat16)
        in_maps.append({
            "xt": xt_c,
            "wq": wq_s, "wk": wk_b, "wv": wv_b, "wo": wo_b,
            "bqt": bqt, "bkt": bkt, "bvr": bvr, "bor": bor,
            "mt0f": zeros0 if hh == 0 else band0,
            "mt0": band0,
            "mt1": band1,
        })
    return in_maps


def kernel(x, Wq, bq, Wk, bk, Wv, bv, Wo, bo):
    if "nc" not in _CACHE:
        _CACHE["nc"] = _build_program()
    nc = _CACHE["nc"]
    in_maps = _prep_inputs(x, Wq, bq, Wk, bk, Wv, bv, Wo, bo)
    res = run_bass_kernel_spmd(nc, in_maps, list(range(NCORES))).results

    output = np.empty((B, T, D), np.float32)
    context = np.empty((B, T, D), np.float32)
    for c in range(NCORES):
        b, hh = c // 2, c % 2
        t0 = hh * TOWN
        output[b, t0:t0 + TOWN] = res[c]["outp"]
        context[b, t0:t0 + TOWN] = res[c]["ctxp"]
    return output, context
